# revision 5
# baseline (speedup 1.0000x reference)
"""Trainium2 Bass kernel for nn_DivMergedLayer1 (dense_mlp, memory-bound).

The baked FFN weights are ultra-sparse: the whole module reduces to
``out = x`` everywhere except four scalars per batch row::

    op   = x[b, 0, 67]                      (opcode channel, >= 0)
    sg   = sum_i f32(f32(60*op) * f32(2^i * x[b, i, 0])) / 60
    s2   = sum_i max((x[b,i,1] > 0.5) * (2^i * x[b,i,1]), exp(-60))
    out[b, 0, k] = x[b,0,k] + f32(60*op * x[b,0,k]) * (-1/60)   k in {2,3,4,5}
    out[b, 0, 2] += sg
    out[b, 0, 5] += op / s2

So the kernel is a memory-bound copy (read 128 MiB + write 128 MiB over
8 cores) with a tiny fused per-row fixup, done while each tile sits in
SBUF. Pure data parallel over the batch axis; 1024 rows per core.
"""

import math

import numpy as np

N_CORES = 8
B, N, D = 8192, 32, 128
F = N * D                  # 4096 flattened features per row
R = B // N_CORES           # 1024 rows per core
P = 128                    # SBUF partitions
QB = 2                     # 128-row blocks per DMA tile (tile = 4 MiB)
T = R // (P * QB)          # DMA tiles per core

OP_COL = 67                # flat index of opcode channel (pos 0, feat 64+3)
SLOT_LO, SLOT_HI = 2, 6    # cleared slots: flat cols 2..5 at position 0

_INV_S = float(np.float32(1.0 / 60.0))
_NEG_INV_S = float(np.float32(-1.0 / 60.0))
_EXP_NEG60 = float(np.float32(math.exp(-60.0)))

_COMPILED = None


def _build():
    import concourse.bacc as bacc
    import concourse.mybir as mybir
    from concourse.tile import TileContext

    f32 = mybir.dt.float32
    mult = mybir.AluOpType.mult
    add = mybir.AluOpType.add
    is_gt = mybir.AluOpType.is_gt
    amax = mybir.AluOpType.max

    nc = bacc.Bacc(
        "TRN2", target_bir_lowering=False, debug=False, num_devices=N_CORES
    )
    x_h = nc.dram_tensor("x", [R, N, D], f32, kind="ExternalInput")
    pw_h = nc.dram_tensor("pw", [P, N], f32, kind="ExternalInput")
    out_h = nc.dram_tensor("out", [R, N, D], f32, kind="ExternalOutput")

    # tile t, partition p holds row t*QB*128 + q*128 + p
    xv = x_h.ap().rearrange("(t q p) n d -> t p q (n d)", p=P, q=QB)
    ov = out_h.ap().rearrange("(t q p) n d -> t p q (n d)", p=P, q=QB)

    with TileContext(nc) as tc:
        with (
            tc.tile_pool(name="const", bufs=1) as cpool,
            tc.tile_pool(name="big", bufs=3) as bpool,
            tc.tile_pool(name="small", bufs=4) as spool,
        ):
            pw = cpool.tile([P, N], f32)
            # SWDGE so the tiny const load doesn't head-block the sync
            # HWDGE ring in front of the first big input DMA.
            nc.gpsimd.dma_start(out=pw[:], in_=pw_h.ap())
            for t in range(T):
                X = bpool.tile([P, QB, F], f32, tag="X")
                nc.sync.dma_start(out=X[:], in_=xv[t])
                for q in range(QB):
                    Bq = X[:, q]
                    Br = Bq.rearrange("p (n d) -> p n d", d=D)
                    a_ap = Br[:, :, 0:1]        # [P, 32] stride-128 view
                    d_ap = Br[:, :, 1:2]
                    op_ap = Bq[:, OP_COL:OP_COL + 1]
                    slots = Bq[:, SLOT_LO:SLOT_HI]

                    op60 = spool.tile([P, 1], f32, tag="op60")
                    g = spool.tile([P, N], f32, tag="g")
                    val = spool.tile([P, N], f32, tag="val")
                    msk = spool.tile([P, N], f32, tag="msk")
                    extra = spool.tile([P, 4], f32, tag="extra")
                    s2 = spool.tile([P, 1], f32, tag="s2")
                    s2r = spool.tile([P, 1], f32, tag="s2r")
                    c4 = spool.tile([P, 4], f32, tag="c4")

                    V = nc.vector
                    V.tensor_scalar_mul(op60[:], op_ap, 60.0)
                    # gather term -> extra[:,0]
                    V.tensor_tensor(g[:], a_ap, pw[:], mult)
                    V.tensor_scalar_mul(g[:], g[:], op60[:])
                    V.tensor_scalar(
                        g[:], g[:], _INV_S, None, mult, add,
                        accum_out=extra[:, 0:1],
                    )
                    # softmax1-reciprocal term -> extra[:,3]
                    V.tensor_tensor(val[:], d_ap, pw[:], mult)
                    V.tensor_scalar(msk[:], d_ap, 0.5, None, is_gt)
                    V.tensor_tensor(val[:], val[:], msk[:], mult)
                    V.tensor_scalar(
                        val[:], val[:], _EXP_NEG60, None, amax, add,
                        accum_out=s2[:],
                    )
                    V.reciprocal(s2r[:], s2[:])
                    V.tensor_tensor(extra[:, 3:4], s2r[:], op_ap, mult)
                    V.memset(extra[:, 1:3], 0.0)
                    # cleared slots, matching the reference's rounding order
                    V.tensor_scalar_mul(c4[:], slots, op60[:])
                    V.scalar_tensor_tensor(c4[:], c4[:], _NEG_INV_S, slots, mult, add)
                    V.tensor_tensor(slots, c4[:], extra[:], add)
                # Output DMAs ride the scalar-engine HWDGE ring so their
                # compute-done waits never stall input prefetch on the sync
                # ring (HWDGE is FIFO per issuing engine).
                nc.scalar.dma_start(out=ov[t], in_=X[:])
    nc.compile()
    return nc


def _get_compiled():
    global _COMPILED
    if _COMPILED is None:
        _COMPILED = _build()
    return _COMPILED


def kernel(**inputs):
    from concourse.bass_utils import run_bass_kernel_spmd

    nc = _get_compiled()
    x = np.ascontiguousarray(np.asarray(inputs["x"], dtype=np.float32))
    assert x.shape == (B, N, D), x.shape
    bpw = np.asarray(inputs["base_powers"]).astype(np.float32)
    pw = np.ascontiguousarray(np.broadcast_to(bpw[None, :], (P, N)))
    in_maps = [
        {"x": np.ascontiguousarray(x[i * R:(i + 1) * R]), "pw": pw}
        for i in range(N_CORES)
    ]
    res = run_bass_kernel_spmd(nc, in_maps, list(range(N_CORES)))
    out = np.concatenate(
        [res.results[i]["out"] for i in range(N_CORES)], axis=0
    )
    return np.ascontiguousarray(out.reshape(B, N, D).astype(np.float32))


# revision 7
# speedup vs baseline: 1.1393x; 1.1393x over previous
"""Trainium2 Bass kernel for nn_DivMergedLayer1 (dense_mlp, memory-bound).

The baked FFN weights are ultra-sparse: the whole module reduces to
``out = x`` everywhere except four scalars per batch row::

    op   = x[b, 0, 67]                      (opcode channel, >= 0)
    sg   = sum_i f32(f32(60*op) * f32(2^i * x[b, i, 0])) / 60
    s2   = sum_i max((x[b,i,1] > 0.5) * (2^i * x[b,i,1]), exp(-60))
    out[b, 0, k] = x[b,0,k] + f32(60*op * x[b,0,k]) * (-1/60)   k in {2,3,4,5}
    out[b, 0, 2] += sg
    out[b, 0, 5] += op / s2

So the kernel is a memory-bound copy (read 128 MiB + write 128 MiB over
8 cores) with a tiny fused per-row fixup, done while each tile sits in
SBUF. Pure data parallel over the batch axis; 1024 rows per core.
"""

import math

import numpy as np

N_CORES = 8
B, N, D = 8192, 32, 128
F = N * D                  # 4096 flattened features per row
R = B // N_CORES           # 1024 rows per core
P = 128                    # SBUF partitions
QB = 2                     # 128-row blocks per DMA tile (tile = 4 MiB)
T = R // (P * QB)          # DMA tiles per core

OP_COL = 67                # flat index of opcode channel (pos 0, feat 64+3)
SLOT_LO, SLOT_HI = 2, 6    # cleared slots: flat cols 2..5 at position 0

_INV_S = float(np.float32(1.0 / 60.0))
_NEG_INV_S = float(np.float32(-1.0 / 60.0))
_EXP_NEG60 = float(np.float32(math.exp(-60.0)))

_COMPILED = None


def _build():
    import concourse.bacc as bacc
    import concourse.mybir as mybir
    from concourse.tile import TileContext

    f32 = mybir.dt.float32
    mult = mybir.AluOpType.mult
    add = mybir.AluOpType.add
    is_gt = mybir.AluOpType.is_gt
    amax = mybir.AluOpType.max

    nc = bacc.Bacc(
        "TRN2", target_bir_lowering=False, debug=False, num_devices=N_CORES
    )
    x_h = nc.dram_tensor("x", [R, N, D], f32, kind="ExternalInput")
    pw_h = nc.dram_tensor("pw", [P, N], f32, kind="ExternalInput")
    out_h = nc.dram_tensor("out", [R, N, D], f32, kind="ExternalOutput")

    # tile t, partition p holds row t*QB*128 + q*128 + p
    xv = x_h.ap().rearrange("(t q p) n d -> t p q (n d)", p=P, q=QB)
    ov = out_h.ap().rearrange("(t q p) n d -> t p q (n d)", p=P, q=QB)

    with TileContext(nc) as tc:
        with (
            tc.tile_pool(name="const", bufs=1) as cpool,
            tc.tile_pool(name="big", bufs=3) as bpool,
            tc.tile_pool(name="small", bufs=4) as spool,
        ):
            pw = cpool.tile([P, N], f32)
            nc.sync.dma_start(out=pw[:], in_=pw_h.ap())
            for t in range(T):
                X = bpool.tile([P, QB, F], f32, tag="X")
                nc.sync.dma_start(out=X[:], in_=xv[t])
                for q in range(QB):
                    Bq = X[:, q]
                    Br = Bq.rearrange("p (n d) -> p n d", d=D)
                    a_ap = Br[:, :, 0:1]        # [P, 32] stride-128 view
                    d_ap = Br[:, :, 1:2]
                    op_ap = Bq[:, OP_COL:OP_COL + 1]
                    slots = Bq[:, SLOT_LO:SLOT_HI]

                    op60 = spool.tile([P, 1], f32, tag="op60")
                    g = spool.tile([P, N], f32, tag="g")
                    val = spool.tile([P, N], f32, tag="val")
                    msk = spool.tile([P, N], f32, tag="msk")
                    extra = spool.tile([P, 4], f32, tag="extra")
                    s2 = spool.tile([P, 1], f32, tag="s2")
                    s2r = spool.tile([P, 1], f32, tag="s2r")
                    c4 = spool.tile([P, 4], f32, tag="c4")

                    V = nc.vector
                    V.tensor_scalar_mul(op60[:], op_ap, 60.0)
                    # gather term -> extra[:,0]
                    V.tensor_tensor(g[:], a_ap, pw[:], mult)
                    V.tensor_scalar_mul(g[:], g[:], op60[:])
                    V.tensor_scalar(
                        g[:], g[:], _INV_S, None, mult, add,
                        accum_out=extra[:, 0:1],
                    )
                    # softmax1-reciprocal term -> extra[:,3]
                    V.tensor_tensor(val[:], d_ap, pw[:], mult)
                    V.tensor_scalar(msk[:], d_ap, 0.5, None, is_gt)
                    V.tensor_tensor(val[:], val[:], msk[:], mult)
                    V.tensor_scalar(
                        val[:], val[:], _EXP_NEG60, None, amax, add,
                        accum_out=s2[:],
                    )
                    V.reciprocal(s2r[:], s2[:])
                    V.tensor_tensor(extra[:, 3:4], s2r[:], op_ap, mult)
                    V.memset(extra[:, 1:3], 0.0)
                    # cleared slots, matching the reference's rounding order
                    V.tensor_scalar_mul(c4[:], slots, op60[:])
                    V.scalar_tensor_tensor(c4[:], c4[:], _NEG_INV_S, slots, mult, add)
                    V.tensor_tensor(slots, c4[:], extra[:], add)
                nc.sync.dma_start(out=ov[t], in_=X[:])
    nc.compile()
    return nc


def _get_compiled():
    global _COMPILED
    if _COMPILED is None:
        _COMPILED = _build()
    return _COMPILED


def kernel(**inputs):
    from concourse.bass_utils import run_bass_kernel_spmd

    nc = _get_compiled()
    x = np.ascontiguousarray(np.asarray(inputs["x"], dtype=np.float32))
    assert x.shape == (B, N, D), x.shape
    bpw = np.asarray(inputs["base_powers"]).astype(np.float32)
    pw = np.ascontiguousarray(np.broadcast_to(bpw[None, :], (P, N)))
    in_maps = [
        {"x": np.ascontiguousarray(x[i * R:(i + 1) * R]), "pw": pw}
        for i in range(N_CORES)
    ]
    res = run_bass_kernel_spmd(nc, in_maps, list(range(N_CORES)))
    out = np.concatenate(
        [res.results[i]["out"] for i in range(N_CORES)], axis=0
    )
    return np.ascontiguousarray(out.reshape(B, N, D).astype(np.float32))
